# revision 43
# baseline (speedup 1.0000x reference)
"""Multi-head causal attention (B=2, S=2048, D=1024, H=16) on 8 NeuronCores.

Sharding: data-parallel over batch (2) x tensor-parallel over head groups
(4 groups of 4 heads).  Each core computes QKV projections for its head
slice, causal attention for its 4 heads, and a partial output projection;
the host sums the 4 head-group partials per batch and adds the bias.

All matmul operands are bf16 (f32 PSUM accumulation), which removes the
fp32r narrow-tile penalty and halves input DMA bytes.  Scores are computed
transposed (keys on partitions) so softmax denominators come from a ones
column appended to V.  Engine roles: PE matmuls only (with a p-state
warmup burst during the initial DMA wait), ACT exp only, DVE evacuations/
reciprocal/mask/normalize, Pool broadcasts + SWDGE DMA, SP HWDGE DMA.
The per-head context is normalized directly out of PSUM (tensor_mul of
the PSUM context rows with the broadcast reciprocal), skipping the
separate evacuation copy.
"""

import numpy as np
import ml_dtypes

import concourse.bacc as bacc
import concourse.mybir as mybir
import concourse.tile as tile
from concourse import bass_utils
from concourse.bass import ds, ts


def ds512(i, lo):
    return ds(i * 512 + lo, 512 - lo)


f32 = mybir.dt.float32
bf16 = mybir.dt.bfloat16
AFT = mybir.ActivationFunctionType

B, S, D, H = 2, 2048, 1024, 16
HD = D // H          # 64
NCORES = 8
NG = 4               # head groups (cores per batch)
GH = H // NG         # heads per core = 4
GO = GH * HD         # output channels per core = 256
KT = D // 128        # 8 k-tiles over the model dim
SC = S // 512        # 4 query chunks of 512
SJ = S // 128        # 16 key tiles of 128
NWARM = 14           # PE p-state warmup matmuls

_CACHE = {}


def _build():
    nc = bacc.Bacc(None)
    xT_d = nc.dram_tensor("xT", [D, S], bf16, kind="ExternalInput")
    wq_d = nc.dram_tensor("wqT", [D, GO], bf16, kind="ExternalInput")
    wk_d = nc.dram_tensor("wkT", [D, GO], bf16, kind="ExternalInput")
    wv_d = nc.dram_tensor("wvT", [D, GO], bf16, kind="ExternalInput")
    wo_d = nc.dram_tensor("woT", [GO, D], bf16, kind="ExternalInput")
    mask_d = nc.dram_tensor("mask", [128, 256], bf16, kind="ExternalInput")
    out_d = nc.dram_tensor("out", [S, D], bf16, kind="ExternalOutput")

    with tile.TileContext(nc) as tc:
        with tc.tile_pool(name="const", bufs=1) as constp, \
             tc.tile_pool(name="big", bufs=1) as bigp, \
             tc.tile_pool(name="probs", bufs=5) as probsp, \
             tc.tile_pool(name="osb", bufs=8) as osbp, \
             tc.tile_pool(name="mm_ps", bufs=2, space="PSUM") as mmps, \
             tc.tile_pool(name="sc_ps", bufs=2, space="PSUM") as scps, \
             tc.tile_pool(name="ctx_ps", bufs=2, space="PSUM") as ctxps:

            wq_t = constp.tile([128, KT, GO], bf16)
            wk_t = constp.tile([128, KT, GO], bf16)
            wv_t = constp.tile([128, KT, GO], bf16)
            wo_t = constp.tile([128, 2, D], bf16)
            mask_t = constp.tile([128, 2, 128], bf16)
            warm_t = constp.tile([128, 128], bf16)
            scratch_t = constp.tile([1, 1], bf16)
            xT_t = bigp.tile([128, KT, S], bf16)
            xT_src = xT_d.rearrange("(k p) s -> p k s", p=128)

            # warmup: ramp the PE p-state during the initial DMA wait, and
            # preload the Exp activation table off the critical path.
            nc.vector.memset(warm_t[:], 0.0)
            nc.scalar.activation(scratch_t[:], warm_t[0:1, 0:1], AFT.Exp)
            for _ in range(NWARM):
                wps = mmps.tile([128, 512], f32, tag="mm", name="wps")
                nc.tensor.matmul(wps[:, 0:128], warm_t[:], warm_t[:],
                                 start=True, stop=True)

            # DMA order: K/Q weights and x chunk 0 first (fine-grained per
            # k-tile slices, alternating SP / gpsimd queues) so the chunk-0
            # projections can start ~4us in; the rest streams behind.
            def dma_x_chunk(c, odd_eng):
                for k in range(KT):
                    eng = nc.sync if k % 2 == 0 else odd_eng
                    eng.dma_start(xT_t[:, k, ts(c, 512)],
                                  xT_src[:, k, ts(c, 512)])

            # x chunks 2/3's odd slices issue from ACT (still exp-light at
            # that point) so Pool is free for the early mask multiplies.
            nc.sync.dma_start(wk_t[:], wk_d.rearrange("(k p) o -> p k o", p=128))
            nc.gpsimd.dma_start(wq_t[:], wq_d.rearrange("(k p) o -> p k o", p=128))
            dma_x_chunk(0, nc.gpsimd)
            nc.gpsimd.dma_start(wv_t[:], wv_d.rearrange("(k p) o -> p k o", p=128))
            dma_x_chunk(1, nc.gpsimd)
            nc.sync.dma_start(mask_t[:], mask_d.rearrange("p (z c) -> p z c", z=2))
            nc.gpsimd.dma_start(wo_t[:], wo_d.rearrange("(t p) n -> p t n", p=128))
            dma_x_chunk(2, nc.scalar)
            dma_x_chunk(3, nc.scalar)

            QT_t = bigp.tile([128, 2, S], bf16)     # [o, s] head-major
            KTr_t = bigp.tile([128, 2, S], bf16)
            V_t = bigp.tile([128, SJ, GH, HD + 1], bf16)  # V cols + ones
            ctxT_t = bigp.tile([128, 2, S], bf16)

            # ones column for the softmax denominators
            nc.vector.memset(V_t[:, :, :, HD], 1.0)

            # --- emission-order-interleaved pipeline ---
            def kq_group(w_t, dst, t, c):
                mm = mmps.tile([128, 512], f32, tag="mm", name="mm")
                for k in range(KT):
                    nc.tensor.matmul(
                        mm[:], w_t[:, k, ts(t, 128)], xT_t[:, k, ts(c, 512)],
                        start=(k == 0), stop=(k == KT - 1))
                # t=0 groups land at chunk boundaries where ACT's exp queue
                # has just drained (run immediately there); t=1 groups run
                # mid-chunk where ACT is busy with exps, so use DVE.
                if t == 0:
                    nc.scalar.copy(dst[:, t, ts(c, 512)], mm[:])
                else:
                    nc.vector.tensor_copy(dst[:, t, ts(c, 512)], mm[:])

            def v_group(jt):
                mm = mmps.tile([128, 512], f32, tag="mm", name="mm")
                for k in range(KT):
                    nc.tensor.matmul(
                        mm[:, 0:GO], xT_t[:, k, ts(jt, 128)], wv_t[:, k, :],
                        start=(k == 0), stop=(k == KT - 1))
                src = mm[:, 0:GO].rearrange("p (h e) -> p h e", e=HD)
                nc.vector.tensor_copy(V_t[:, jt, :, 0:HD], src)

            def attn_pair(i, hp, fine_norm=False, interleave=()):
                # heads h0 = 2*hp, h1 = 2*hp+1 share QT/KT tile t=hp with
                # partition offsets 0 and 64.  `interleave` is a list of
                # thunks (projection / next-chunk prep groups) paced into
                # the tile stream so the PE-only work runs while ACT chews
                # through the exp backlog instead of idling afterwards.
                to = hp
                heads = (2 * hp, 2 * hp + 1)
                cps = [ctxps.tile([128, 512], f32, tag="ctx", name="cps")
                       for _ in heads]
                njt = 4 * i + 4
                LAG = 5  # scores/exp run this many j-tiles ahead of ctx

                pending = {}

                def emit_scores(jt):
                    r = jt - 4 * i
                    lo = max(r, 0) * 128
                    prp = probsp.tile([128, 2, 512], bf16, tag="pr", name="prp",
                                      bufs=7)
                    scp = scps.tile([128, 2, 512], f32, tag="sc", name="scp")
                    for z, h in enumerate(heads):
                        po = 64 * z
                        nc.tensor.matmul(
                            scp[:, z, lo:512],
                            KTr_t[po:po + 64, to, ts(jt, 128)],
                            QT_t[po:po + 64, to, ds512(i, lo)],
                            start=True, stop=True)
                    nc.scalar.activation(prp[:, :, lo:512], scp[:, :, lo:512],
                                         AFT.Exp)
                    if r >= 0:
                        nc.gpsimd.tensor_mul(
                            prp[:, :, lo:lo + 128], prp[:, :, lo:lo + 128],
                            mask_t[:])
                    pending[jt] = (lo, prp)

                def emit_ctx(jt):
                    lo, prp = pending.pop(jt)
                    # fine_norm closes the accumulation group two tiles
                    # early (columns [0, 256) take no further writes after
                    # tile 4i+1) so the first normalize half can read PSUM
                    # while the remaining tiles still accumulate the rest.
                    if fine_norm:
                        stop = (jt == 4 * i + 1)
                        skip = (jt > 4 * i + 1)
                    else:
                        stop = (jt == njt - 1)
                        skip = False
                    for z, h in enumerate(heads):
                        nc.tensor.matmul(
                            cps[z][0:HD + 1, lo:512], V_t[:, jt, h, :],
                            prp[:, z, lo:512],
                            start=(jt == 0), stop=stop,
                            skip_group_check=skip)

                done = 0
                for jt in range(njt):
                    emit_scores(jt)
                    want = (jt + 1) * len(interleave) // njt
                    while done < want:
                        interleave[done]()
                        done += 1
                    if jt >= LAG:
                        emit_ctx(jt - LAG)
                while done < len(interleave):
                    interleave[done]()
                    done += 1
                for jt in range(max(0, njt - LAG), njt):
                    emit_ctx(jt)
                    if fine_norm and jt == 4 * i + 1:
                        # ctx columns [0, 256) take no further writes after
                        # this tile — normalize the first half early so the
                        # tail projections of m-tiles 4i..4i+1 can start
                        # before the pair fully drains.
                        norm_half(i, hp, cps, 0)
                if fine_norm:
                    norm_half(i, hp, cps, 256)
                    return lambda: None
                return lambda: finish_pair(i, hp, cps)

            def norm_half(i, hp, cps, off):
                to = hp
                for z in range(2):
                    po = 64 * z
                    rec = probsp.tile([1, 256], f32, tag="rech", name="rec",
                                      bufs=2)
                    nc.vector.reciprocal(
                        rec[:], cps[z][HD:HD + 1, ds(off, 256)])
                    bcs = probsp.tile([HD, 256], f32, tag="bcsh", name="bcs",
                                      bufs=2)
                    nc.gpsimd.partition_broadcast(bcs[:], rec[:])
                    nc.vector.tensor_mul(
                        ctxT_t[po:po + 64, to, ds(i * 512 + off, 256)],
                        cps[z][0:HD, ds(off, 256)], bcs[:])

            def finish_pair(i, hp, cps):
                fine_norm = False
                to = hp
                heads = (2 * hp, 2 * hp + 1)
                # normalize straight out of PSUM: reciprocal of the
                # denominator row, partition-broadcast, then tensor_mul
                # PSUM x bcast -> bf16 ctxT.  fine_norm splits the multiply
                # per m-tile (z-interleaved) so the tail projection can
                # start as soon as its first columns are normalized.
                recs, bcss = [], []
                for z, h in enumerate(heads):
                    rec = probsp.tile([1, 512], f32, tag="rec", name="rec",
                                      bufs=2)
                    nc.vector.reciprocal(rec[:], cps[z][HD:HD + 1, :])
                    bcs = probsp.tile([HD, 512], f32, tag="bcs", name="bcs",
                                      bufs=2)
                    nc.gpsimd.partition_broadcast(bcs[:], rec[:])
                    recs.append(rec)
                    bcss.append(bcs)
                if fine_norm:
                    for q in range(4):
                        for z, h in enumerate(heads):
                            po = 64 * z
                            nc.vector.tensor_mul(
                                ctxT_t[po:po + 64, to,
                                       ds(i * 512 + q * 128, 128)],
                                cps[z][0:HD, ts(q, 128)],
                                bcss[z][:, ts(q, 128)])
                else:
                    for z, h in enumerate(heads):
                        po = 64 * z
                        nc.vector.tensor_mul(
                            ctxT_t[po:po + 64, to, ts(i, 512)],
                            cps[z][0:HD, :], bcss[z][:])

            def proj_group(m, n, last=False):
                if last:
                    pool, tg = ((mmps, "mm"), (scps, "sc"), (ctxps, "ctx"))[
                        (2 * m + n) % 3]
                else:
                    pool, tg = mmps, "mm"
                mm = pool.tile([128, 512], f32, tag=tg, name="mm")
                for t in range(2):
                    nc.tensor.matmul(
                        mm[:], ctxT_t[:, t, ts(m, 128)], wo_t[:, t, ts(n, 512)],
                        start=(t == 0), stop=(t == 1))
                ot = osbp.tile([128, 512], bf16, tag="ot", name="ot")
                # only ACT and DVE may read PSUM; keep ACT exp-pure except
                # at the tail where exp is already done.
                if last and (m + n) % 2 == 1:
                    nc.scalar.copy(ot[:], mm[:])
                else:
                    nc.vector.tensor_copy(ot[:], mm[:])
                if last:
                    deng = (nc.sync, nc.scalar, nc.gpsimd)[(2 * m + n) % 3]
                else:
                    deng = nc.sync
                deng.dma_start(out_d[ts(m, 128), ts(n, 512)], ot[:])

            def prep_units(i):
                units = [
                    lambda: kq_group(wk_t, KTr_t, 0, i),
                    lambda: kq_group(wq_t, QT_t, 0, i),
                    lambda: kq_group(wk_t, KTr_t, 1, i),
                    lambda: kq_group(wq_t, QT_t, 1, i),
                ]
                for jt in range(4 * i, 4 * i + 4):
                    units.append(lambda jt=jt: v_group(jt))
                return units

            def proj_units(i, last=False):
                units = []
                for m in range(4 * i, 4 * i + 4):
                    for n in range(2):
                        units.append(
                            lambda m=m, n=n: proj_group(m, n, last=last))
                return units

            for u in prep_units(0):
                u()
            held = []
            for i in range(SC):
                inter0 = proj_units(i - 1) if i > 0 else []
                if i == SC - 1:
                    # hold a few previous-chunk projection groups back so
                    # the PE has work during the final normalize chain.
                    held = inter0[4:]
                    inter0 = inter0[:4]
                fin0 = attn_pair(i, 0, fine_norm=True, interleave=inter0)
                fin0()
                fin1 = attn_pair(
                    i, 1, fine_norm=True,
                    interleave=prep_units(i + 1) if i + 1 < SC else ())
                fin1()
            for u in held:
                u()
            for u in proj_units(SC - 1, last=True):
                u()

    nc.compile()
    return nc


def _causal_mask():
    p = np.arange(128)[:, None]
    c = np.arange(128)[None, :]
    m = (p <= c).astype(np.float32)
    return np.concatenate([m, m], axis=1)


def _bf(a):
    return np.ascontiguousarray(a).astype(ml_dtypes.bfloat16)


def _core_inputs(x, Wq, Wk, Wv, Wo, mask, core):
    b, g = divmod(core, NG)
    sl = slice(g * GO, (g + 1) * GO)
    return {
        "xT": _bf(x[b].T),
        "wqT": _bf((Wq[sl, :] / np.sqrt(HD)).T),
        "wkT": _bf(Wk[sl, :].T),
        "wvT": _bf(Wv[sl, :].T),
        "woT": _bf(Wo[:, sl].T),
        "mask": _bf(mask),
    }


def kernel(x, Wq, Wk, Wv, Wo, bo):
    x = np.asarray(x, dtype=np.float32)
    Wq = np.asarray(Wq, dtype=np.float32)
    Wk = np.asarray(Wk, dtype=np.float32)
    Wv = np.asarray(Wv, dtype=np.float32)
    Wo = np.asarray(Wo, dtype=np.float32)
    bo = np.asarray(bo, dtype=np.float32)

    if "nc" not in _CACHE:
        _CACHE["nc"] = _build()
    nc = _CACHE["nc"]

    mask = _causal_mask()
    in_maps = [_core_inputs(x, Wq, Wk, Wv, Wo, mask, core)
               for core in range(NCORES)]

    res = bass_utils.run_bass_kernel_spmd(nc, in_maps, core_ids=list(range(NCORES)))
    _CACHE["last_result"] = res

    out = np.zeros((B, S, D), dtype=np.float32)
    for core in range(NCORES):
        b = core // NG
        out[b] += np.asarray(res.results[core]["out"]).astype(np.float32)
    out += bo
    return out
